# revision 36
# baseline (speedup 1.0000x reference)
"""Trainium2 Bass kernel for causal self-attention with cumulative-phase rotary
embedding (nn_CausalSelfAttention_64338610094602).

Sharding: 8 cores = 4 batches x 2 head-groups (tensor-parallel over heads).
Each core computes, for its (batch, 8-head group), per head:
  QKV projections, rotation + RMSNorm, causal attention (transposed-scores
  layout, max-free softmax), and a partial output projection. Host sums the
  two head-group partials per batch.

The tiny omega/cumsum/trig path is computed on host (0.016% of FLOPs) and
shipped as cos/sin tables. Score-path GEMMs run in float32r; the probs/V/Wo
side runs in bf16 (rel-err budget ~5e-3 vs 2e-2 tolerance).

Key scheduling ideas vs the previous version:
  - PSUM projection banks are released by a single Pool-engine eviction copy;
    rotation (4 fused DVE ops via duplicated cos/sin tables) reads the SBUF
    copy off the critical path and writes q/k tiles directly (f32r cast).
  - k-side rstd (and the 1/sqrt(D) scale, and both gammas via host-folded
    Wk) ride the exp's per-partition scale, via a PE-transposed rstd column
    table; only the q side needs broadcast-normalize.
  - softmax is software-pipelined 2 blocks deep; denominators are packed
    pairwise (DVE bf16 adds) to halve the PE's ones-matmuls.
  - V is computed d-major at full moving width then PE-transposed to
    token-major bf16.
  - P3 reloads y/Wo (bf16) with DMA prefetched under the last head's
    attention, after the x^T buffer is freed.
"""
import math

import numpy as np
import ml_dtypes

import concourse.mybir as mybir
import concourse.tile as tile
from concourse import bacc
from concourse.bass_utils import run_bass_kernel_spmd

B, T, C = 4, 2048, 2048
H, D, DH = 16, 128, 64
HG = 8          # heads per core (head-group)
GD = HG * D     # group output dims = 1024
NT = T // 512   # 4 J-blocks of 512
NCT = C // 128  # 16 contraction tiles
EPS = 1e-5
SCL = 1.0 / math.sqrt(D)
OMEGA_SCALE = 16.0

dt = mybir.dt
AF = mybir.ActivationFunctionType
ALU = mybir.AluOpType

_CACHE = {}


def _round_f32r(x):
    """Round fp32 array to float32r (13-bit mantissa, round-to-nearest-even)."""
    x = np.ascontiguousarray(x, dtype=np.float32)
    b = x.view(np.uint32).copy()
    low = b & np.uint32(0x3FF)
    bb = b & ~np.uint32(0x3FF)
    rnd = (low > 0x200) | ((low == 0x200) & (((bb >> 10) & 1) == 1))
    return (bb + (rnd.astype(np.uint32) << 10)).view(np.float32)


def _build():
    f32, f32r, bf16 = dt.float32, dt.float32r, dt.bfloat16
    nc = bacc.Bacc(None, target_bir_lowering=False)
    with tile.TileContext(nc) as tc:
        xt_d = nc.dram_tensor("xt", (C, T), f32r, kind="ExternalInput")
        wq_d = nc.dram_tensor("wq", (C, GD), f32r, kind="ExternalInput")
        wk_d = nc.dram_tensor("wk", (C, GD), f32r, kind="ExternalInput")
        wv_d = nc.dram_tensor("wv", (C, GD), f32r, kind="ExternalInput")
        wo_d = nc.dram_tensor("wo", (GD, C), bf16, kind="ExternalInput")
        trigc_d = nc.dram_tensor("trigc", (128, T), f32, kind="ExternalInput")
        trigs_d = nc.dram_tensor("trigs", (128, T), f32, kind="ExternalInput")
        masks_d = nc.dram_tensor("masks", (128, 4 * 512), bf16, kind="ExternalInput")
        onesAb_d = nc.dram_tensor("onesAb", (128, 1), bf16, kind="ExternalInput")
        onesB_d = nc.dram_tensor("onesB", (1, 128), f32r, kind="ExternalInput")
        idb_d = nc.dram_tensor("idb", (128, 128), bf16, kind="ExternalInput")
        idr_d = nc.dram_tensor("idr", (128, 128), f32r, kind="ExternalInput")
        wcolq_d = nc.dram_tensor("wcolq", (128, 15), bf16, kind="ExternalInput")
        wcolk_d = nc.dram_tensor("wcolk", (128, 15), bf16, kind="ExternalInput")
        bias8_d = nc.dram_tensor("bias8", (8, 1), f32, kind="ExternalInput")
        out_d = nc.dram_tensor("out", (T, C), f32, kind="ExternalOutput")

        with tc.tile_pool(name="dram", bufs=1, space="DRAM") as dramp:
            yspill = dramp.tile([128, 7 * T], bf16)  # heads 0-6 yT

            with tc.tile_pool(name="const", bufs=1) as constp:
                trigc = constp.tile([128, T], f32)
                nc.sync.dma_start(trigc[:], trigc_d[:])
                trigs = constp.tile([128, T], f32)
                nc.sync.dma_start(trigs[:], trigs_d[:])
                masks = constp.tile([128, 4 * 512], bf16)
                nc.sync.dma_start(masks[:], masks_d[:])
                onesAb = constp.tile([128, 1], bf16)
                nc.sync.dma_start(onesAb[:], onesAb_d[:])
                onesB = constp.tile([1, 128], f32r)
                nc.sync.dma_start(onesB[:], onesB_d[:])
                idb = constp.tile([128, 128], bf16)
                nc.sync.dma_start(idb[:], idb_d[:])
                idr = constp.tile([128, 128], f32r)
                nc.sync.dma_start(idr[:], idr_d[:])
                wcolq = constp.tile([128, 15], bf16)
                nc.sync.dma_start(wcolq[:], wcolq_d[:])
                wcolk = constp.tile([128, 15], bf16)
                nc.sync.dma_start(wcolk[:], wcolk_d[:])
                bias8 = constp.tile([8, 1], f32)
                nc.sync.dma_start(bias8[:], bias8_d[:])
                eps8 = constp.tile([8, 1], f32)
                nc.vector.memset(eps8[:], EPS)

                with tc.tile_pool(name="qkv", bufs=1) as qkvp, \
                     tc.tile_pool(name="wst", bufs=3) as wst, \
                     tc.tile_pool(name="wk512", bufs=1) as wkp, \
                     tc.tile_pool(name="exp", bufs=1) as expool, \
                     tc.tile_pool(name="rows", bufs=1) as rowsp:
                    env = dict(
                        nc=nc, tc=tc, wst=wst, wkp=wkp, expool=expool,
                        rowsp=rowsp, qkvp=qkvp, trigc=trigc, trigs=trigs,
                        masks=masks, onesAb=onesAb, onesB=onesB, idb=idb,
                        idr=idr, wcolq=wcolq, wcolk=wcolk, bias8=bias8,
                        eps8=eps8, wq_d=wq_d, wk_d=wk_d, wv_d=wv_d,
                        yspill=yspill)

                    with tc.tile_pool(name="xtp", bufs=1) as xtp:
                        xts = xtp.tile([128, NCT * T], f32r)
                        for i in range(NCT):
                            nc.sync.dma_start(xts[:, i * T:(i + 1) * T],
                                              xt_d[i * 128:(i + 1) * 128, :])
                        env["xts"] = xts
                        heads = {}
                        for h in range(HG):
                            heads[h] = _head_qkv(env, h)
                            if h < HG - 1:
                                _head_attn(env, h, heads[h], None)
                    # xts freed; prefetch P3 operands under head 7's attention
                    with tc.tile_pool(name="p3", bufs=1) as p3p, \
                         tc.tile_pool(name="p3o", bufs=1) as p3o:
                        yall = p3p.tile([128, HG * T], bf16)
                        wosb = p3p.tile([128, HG * C], bf16)
                        for h in range(7):
                            nc.sync.dma_start(yall[:, h * T:(h + 1) * T],
                                              yspill[:, h * T:(h + 1) * T])
                        for h in range(HG):
                            nc.sync.dma_start(wosb[:, h * C:(h + 1) * C],
                                              wo_d[h * 128:(h + 1) * 128, :])
                        _head_attn(env, 7, heads[7], yall)

                        with tc.tile_pool(name="p3ps", bufs=1,
                                          space="PSUM") as p3ps:
                            for ti in range(T // 128):
                                ops = [p3ps.tile([128, 512], f32, tag=f"c{cb}",
                                                 bufs=2,
                                                 name=f"op_{ti}_{cb}")
                                       for cb in range(4)]
                                for hh in range(HG):
                                    for cb in range(4):
                                        nc.tensor.matmul(
                                            ops[cb][:],
                                            yall[:, hh * T + ti * 128:
                                                 hh * T + (ti + 1) * 128],
                                            wosb[:, hh * C + cb * 512:
                                                 hh * C + (cb + 1) * 512],
                                            start=(hh == 0), stop=(hh == HG - 1))
                                for cb in range(4):
                                    osb = p3o.tile([128, 512], f32,
                                                   tag=f"osb{cb % 2}",
                                                   bufs=2,
                                                   name=f"osb_{ti}_{cb}")
                                    if cb % 2:
                                        nc.vector.tensor_copy(osb[:], ops[cb][:])
                                    else:
                                        nc.scalar.copy(osb[:], ops[cb][:])
                                    nc.sync.dma_start(
                                        out_d[ti * 128:(ti + 1) * 128,
                                              cb * 512:(cb + 1) * 512],
                                        osb[:])
    nc.compile()
    return nc


def _head_qkv(env, h):
    """Fused V+Q streaming projections, tails, K pass, ssq/rstd for head h."""
    nc = env["nc"]
    tc = env["tc"]
    xts, wst, wkp, qkvp = env["xts"], env["wst"], env["wkp"], env["qkvp"]
    f32, f32r, bf16 = dt.float32, dt.float32r, dt.bfloat16

    q_sb = qkvp.tile([128, T], f32r, tag="q", name=f"q_{h}")
    k_sb = qkvp.tile([128, T], f32r, tag="k", name=f"k_{h}")
    v_sb = qkvp.tile([128, T], bf16, tag="v", name=f"v_{h}")
    vT = qkvp.tile([128, T], bf16, tag="k", name=f"vT_{h}")  # shares k slot
    rstdT = qkvp.tile([128, 16], f32, tag="rT", name=f"rstdT_{h}")

    def proj_tail(J, qps, dest, evict_only=False, site=None):
        """Evict bank J (DVE), square into sq_J (ACT), rotate into dest."""
        qtmp = wkp.tile([128, 512], f32, tag=f"qt{J}", name=f"qtmp_{h}_{J}")
        nc.vector.tensor_copy(qtmp[:], qps[J][:])  # frees bank a{J}
        if evict_only:
            return qtmp
        sq = wkp.tile([128, 512], bf16, tag=f"sq{J}", name=f"sq_{h}_{site}")
        nc.scalar.activation(sq[:], qtmp[:], AF.Square)
        _rotate(env, qtmp, dest, J)
        return sq

    def krotate(J, qtmp, dest):
        _rotate(env, qtmp, dest, J)

    with tc.tile_pool(name=f"psa_{h}", bufs=1, space="PSUM") as psa:
        # ---- fused stream: V (d-major) + Q projections ride the x DMA ----
        with tc.tile_pool(name=f"psv_{h}", bufs=1, space="PSUM") as psv:
            vps, qps = {}, {}
            for J in range(NT):
                vps[J] = psv.tile([128, 512], f32, tag=f"v{J}",
                                  name=f"vp_{h}_{J}")
                qps[J] = psa.tile([128, 512], f32, tag=f"a{J}",
                                  name=f"qp_{h}_{J}")
            for i in range(NCT):
                wvt = wst.tile([128, 128], f32r, tag="w", name=f"wv_{h}_{i}")
                nc.sync.dma_start(wvt[:],
                                  env["wv_d"][i * 128:(i + 1) * 128,
                                              h * 128:(h + 1) * 128])
                wqt = wst.tile([128, 128], f32r, tag="w", name=f"wq_{h}_{i}")
                nc.sync.dma_start(wqt[:],
                                  env["wq_d"][i * 128:(i + 1) * 128,
                                              h * 128:(h + 1) * 128])
                for J in range(NT):
                    sl = slice(i * T + J * 512, i * T + J * 512 + 512)
                    nc.tensor.matmul(vps[J][:], wvt[:], xts[:, sl],
                                     start=(i == 0), stop=(i == NCT - 1))
                    nc.tensor.matmul(qps[J][:], wqt[:], xts[:, sl],
                                     start=(i == 0), stop=(i == NCT - 1))
            for J in range(NT):
                nc.scalar.copy(vT[:, J * 512:(J + 1) * 512], vps[J][:])
        # ---- q tails (evict, square, rotate) while transposes/K run ----
        sqq = [proj_tail(J, qps, q_sb, site=4 + J) for J in range(NT)]
        # ---- V transposes to token-major bf16 ----
        with tc.tile_pool(name=f"pst_{h}", bufs=2, space="PSUM") as pst:
            for tq in range(4):
                tp = pst.tile([128, 512], bf16, tag="tp", name=f"tp_{h}_{tq}")
                for k in range(4):
                    tt = tq * 4 + k
                    nc.tensor.transpose(tp[:, k * 128:(k + 1) * 128],
                                        vT[:, tt * 128:(tt + 1) * 128],
                                        env["idb"][:])
                nc.scalar.copy(v_sb[:, tq * 512:(tq + 1) * 512], tp[:])
        # ---- K pass (reuses the a{J} banks) ----
        kps = {}
        for J in range(NT):
            kps[J] = psa.tile([128, 512], f32, tag=f"a{J}",
                              name=f"kp_{h}_{J}")
        for i in range(NCT):
            wkt = wst.tile([128, 128], f32r, tag="w", name=f"wk_{h}_{i}")
            nc.sync.dma_start(wkt[:],
                              env["wk_d"][i * 128:(i + 1) * 128,
                                          h * 128:(h + 1) * 128])
            for J in range(NT):
                nc.tensor.matmul(
                    kps[J][:], wkt[:],
                    xts[:, i * T + J * 512:i * T + J * 512 + 512],
                    start=(i == 0), stop=(i == NCT - 1))
        with tc.tile_pool(name=f"pss_{h}", bufs=1, space="PSUM") as pss:
            ssqps = pss.tile([8, 512], f32, name=f"ssqps_{h}")
            # q ssq (squares finished during the K pass); q sites rows 4-7
            for J in range(NT):
                site = 4 + J
                nc.tensor.matmul(ssqps[:],
                                 env["wcolq"][:, 7 - site:15 - site],
                                 sqq[J][:], start=(J == 0), stop=False)
            # k evicts first (free banks), then squares + ssq; rotates later
            ktmp = [proj_tail(J, kps, k_sb, evict_only=True)
                    for J in range(NT)]
            for J in range(NT):
                sqk = wkp.tile([128, 512], bf16, tag=f"sq{J}",
                               name=f"sqk_{h}_{J}")
                nc.scalar.activation(sqk[:], ktmp[J][:], AF.Square)
                nc.tensor.matmul(ssqps[:],
                                 env["wcolk"][:, 7 - J:15 - J],
                                 sqk[:], start=False, stop=(J == NT - 1))
            # rstd: k rows 0-3 get ln(SCL) bias; q rows 4-7 plain
            lnt = wkp.tile([8, 512], f32, tag="tcc", name=f"lnt_{h}")
            nc.scalar.activation(lnt[:], ssqps[:], AF.Ln, scale=1.0 / 128.0,
                                 bias=env["eps8"][:])
            rstd = wkp.tile([8, 512], f32r, tag="rstd8", name=f"rstd_{h}")
            nc.scalar.activation(rstd[:], lnt[:], AF.Exp, scale=-0.5,
                                 bias=env["bias8"][:])
            # transpose k rows into per-key columns for the exp scale
            with tc.tile_pool(name=f"psr_{h}", bufs=1, space="PSUM") as psr:
                rt = psr.tile([128, 16], f32r, name=f"rt_{h}")
                for c in range(4):
                    nc.tensor.transpose(rt[:, c * 4:(c + 1) * 4],
                                        rstd[0:4, c * 128:(c + 1) * 128],
                                        env["idr"][0:4, 0:4])
                nc.vector.tensor_copy(rstdT[:], rt[:])
            # k rotations (consumed gradually by 2c)
            for J in range(NT):
                krotate(J, ktmp[J], k_sb)
    return q_sb, k_sb, v_sb, rstdT, rstd


def _rotate(env, qtmp, dest, J):
    """rot1 = q1 cos + q2 sin ; rot2 = q2 cos - q1 sin.

    DVE inputs must share a start partition; the cross-half sin products
    ride the (legal) shifted writes of tsw. The sub runs on Pool."""
    nc = env["nc"]
    wkp = env["wkp"]
    f32 = dt.float32
    sl = slice(J * 512, (J + 1) * 512)
    tcc = wkp.tile([128, 512], f32, tag="tcc", name="tcc_t")
    tsw = wkp.tile([128, 512], f32, tag="tss", name="tsw_t")
    nc.vector.tensor_tensor(tcc[:], qtmp[:], env["trigc"][:, sl],
                            op=ALU.mult)
    nc.vector.tensor_tensor(tsw[0:DH, :], qtmp[DH:128, :],
                            env["trigs"][DH:128, sl], op=ALU.mult)
    nc.vector.tensor_tensor(tsw[DH:128, :], qtmp[0:DH, :],
                            env["trigs"][0:DH, sl], op=ALU.mult)
    nc.vector.tensor_add(dest[0:DH, sl], tcc[0:DH, :], tsw[0:DH, :])
    nc.gpsimd.tensor_sub(dest[DH:128, sl], tcc[DH:128, :], tsw[DH:128, :])
def _head_attn(env, h, qkv, yall):
    """2c: causal attention for head h (transposed scores, pipelined)."""
    nc = env["nc"]
    tc = env["tc"]
    q_sb, k_sb, v_sb, rstdT, rstd = qkv
    expool, wkp, rowsp = env["expool"], env["wkp"], env["rowsp"]
    masks, onesAb, onesB = env["masks"], env["onesAb"], env["onesB"]
    f32, f32r, bf16 = dt.float32, dt.float32r, dt.bfloat16

    with tc.tile_pool(name=f"psc_{h}", bufs=1, space="PSUM") as psc:
        pend = [None]

        def epilogue(yps, dps, J):
            rcf = wkp.tile([1, 512], f32, tag="tcc", name=f"rcf_{h}_{J}")
            nc.vector.reciprocal_approx_fast(out=rcf[:], in_=dps[:])
            recip = rowsp.tile([1, 512], f32r, tag="rp", name=f"recip_{h}_{J}")
            nc.vector.tensor_copy(recip[:], rcf[:])
            rbps = psc.tile([128, 512], f32, tag="rb", bufs=1,
                            name=f"rbps_{h}_{J}")
            nc.tensor.matmul(rbps[:], onesB[:], recip[:], start=True, stop=True)
            rbsb = wkp.tile([128, 512], f32, tag="rbsb", name=f"rbsb_{h}_{J}")
            nc.vector.tensor_copy(rbsb[:], rbps[:])
            sl = slice(J * 512, (J + 1) * 512)
            if yall is None:
                yt = expool.tile([128, 512], bf16, tag="yt", name=f"yt_{h}_{J}")
                nc.vector.tensor_tensor(yt[:], yps[:], rbsb[:], op=ALU.mult)
                nc.sync.dma_start(
                    env["yspill"][:, h * T + J * 512:h * T + (J + 1) * 512],
                    yt[:])
            else:
                nc.vector.tensor_tensor(yall[:, 7 * T + J * 512:
                                             7 * T + (J + 1) * 512],
                                        yps[:], rbsb[:], op=ALU.mult)

        for J in range(NT):
            nI = 4 * J + 4
            # JIT q-normalize for this J (broadcast rstd row over partitions)
            sl = slice(J * 512, (J + 1) * 512)
            rrow = rowsp.tile([1, 512], f32r, tag="r0", name=f"rrow_{h}_{J}")
            nc.sync.dma_start(rrow[:], rstd[4 + J:5 + J, :])
            rbn = psc.tile([128, 512], f32, tag="rb", bufs=1,
                           name=f"rbn_{h}_{J}")
            nc.tensor.matmul(rbn[:], onesB[:], rrow[:], start=True, stop=True)
            nc.vector.tensor_tensor(q_sb[:, sl], q_sb[:, sl], rbn[:],
                                    op=ALU.mult)

            yps = psc.tile([128, 512], f32, tag="y", bufs=2,
                           name=f"yps_{h}_{J}")
            dps = psc.tile([1, 512], f32, tag="d", bufs=2,
                           name=f"dps_{h}_{J}")
            uses = []
            pairs = []
            quads = []

            def emit_y(I, yps=yps, dps=dps, nI=nI, uses=uses, quads=quads):
                nc.tensor.matmul(
                    yps[:], v_sb[:, I * 128:(I + 1) * 128], uses[I][:],
                    start=(I == 0), stop=(I == nI - 1))
                if I % 4 == 3:
                    m = I // 4
                    nc.tensor.matmul(dps[:], onesAb[:], quads[m][:],
                                     start=(m == 0), stop=(m == nI // 4 - 1))

            for I in range(nI):
                sps = psc.tile([128, 512], f32, tag="s", bufs=3,
                               name=f"sps_{h}_{J}_{I}")
                nc.tensor.matmul(
                    sps[:], k_sb[:, I * 128:(I + 1) * 128],
                    q_sb[:, J * 512:(J + 1) * 512], start=True, stop=True)
                ex = expool.tile([128, 512], bf16, tag=f"ex{I % 3}",
                                 name=f"ex_{h}_{J}_{I}")
                col = (I % 4) * 4 + (I // 4)
                nc.scalar.activation(ex[:], sps[:], AF.Exp,
                                     scale=rstdT[:, col:col + 1])
                if I >= 4 * J:
                    r = I - 4 * J
                    exm = expool.tile([128, 512], bf16, tag=f"mk{I % 3}",
                                      name=f"exm_{h}_{J}_{I}")
                    nc.vector.tensor_tensor(
                        exm[:], ex[:], masks[:, r * 512:(r + 1) * 512],
                        op=ALU.mult)
                    uses.append(exm)
                else:
                    uses.append(ex)
                if I % 2 == 1:
                    pm = expool.tile([128, 512], bf16, tag=f"ps{(I // 2) % 2}",
                                     name=f"pm_{h}_{J}_{I}")
                    nc.vector.tensor_add(pm[:], uses[I - 1][:], uses[I][:])
                    pairs.append(pm)
                if I % 4 == 3:
                    qd = expool.tile([128, 512], bf16, tag=f"qd{(I // 4) % 2}",
                                     name=f"qd_{h}_{J}_{I}")
                    nc.vector.tensor_add(qd[:], pairs[I // 2 - 1][:],
                                         pairs[I // 2][:])
                    quads.append(qd)
                if I == 1 and pend[0] is not None:
                    pend[0]()
                    pend[0] = None
                if I >= 2:
                    emit_y(I - 2)
            emit_y(nI - 2)
            emit_y(nI - 1)
            pend[0] = (lambda yps=yps, dps=dps, J=J: epilogue(yps, dps, J))
        pend[0]()
        pend[0] = None


def _host_prep(inputs):
    x = np.asarray(inputs["x"], dtype=np.float32)
    Wq = np.asarray(inputs["Wq"], dtype=np.float32)
    Wk = np.asarray(inputs["Wk"], dtype=np.float32)
    Wv = np.asarray(inputs["Wv"], dtype=np.float32)
    Wo = np.asarray(inputs["Wo"], dtype=np.float32)
    w_omega = np.asarray(inputs["w_omega"], dtype=np.float32)
    b_omega = np.asarray(inputs["b_omega"], dtype=np.float32)
    log_freq = np.asarray(inputs["log_freq"], dtype=np.float32)
    q_gamma = np.asarray(inputs["q_gamma"], dtype=np.float32)
    k_gamma = np.asarray(inputs["k_gamma"], dtype=np.float32)

    # host trig path (tiny): omega -> phi -> cos/sin tables per batch
    z = (x.reshape(B * T, C) @ w_omega.reshape(C).astype(np.float64)
         ).reshape(B, T) + float(b_omega[0])
    omega = 1.0 / (1.0 + np.exp(-z.astype(np.float64) / OMEGA_SCALE))
    phi = np.cumsum(omega, axis=1) - omega                       # (B,T)
    freq = np.exp(log_freq.astype(np.float64))                   # (DH,)
    ang = phi[:, None, :] * freq[:, None]                        # (B,DH,T)
    cosb = np.cos(ang).astype(np.float32)
    sinb = np.sin(ang).astype(np.float32)
    trigc = np.concatenate([cosb, cosb], axis=1)                 # (B,128,T)
    trigs = np.concatenate([sinb, sinb], axis=1)

    g2 = (q_gamma * k_gamma).astype(np.float32)                  # (128,)
    wk_scale = np.tile(g2, HG)                                   # (GD per group)
    inv_g2sq = np.zeros(128, dtype=np.float32)
    nz = np.abs(g2) > 1e-12
    inv_g2sq[nz] = 1.0 / (g2[nz] * g2[nz])

    p = np.arange(128)[:, None]
    c = np.arange(512)[None, :]
    masks = np.concatenate(
        [((p + r * 128) <= c).astype(np.float32) for r in range(4)], axis=1
    ).astype(ml_dtypes.bfloat16)
    onesAb = np.ones((128, 1), dtype=ml_dtypes.bfloat16)
    onesB = np.ones((1, 128), dtype=np.float32)
    idb = np.eye(128, dtype=ml_dtypes.bfloat16)
    idr = np.eye(128, dtype=np.float32)
    wcolq = np.zeros((128, 15), dtype=np.float32)
    wcolq[:, 7] = 1.0
    wcolq = wcolq.astype(ml_dtypes.bfloat16)
    wcolk = np.zeros((128, 15), dtype=np.float32)
    wcolk[:, 7] = inv_g2sq
    wcolk = wcolk.astype(ml_dtypes.bfloat16)
    bias8 = np.zeros((8, 1), dtype=np.float32)
    bias8[0:4] = math.log(SCL)  # k sites occupy rows 0-3

    in_maps = []
    for core in range(8):
        b, g = core // 2, core % 2
        wk_g = Wk[g * GD:(g + 1) * GD, :] * wk_scale[:, None]
        in_maps.append({
            "xt": _round_f32r(x[b].T),
            "wq": _round_f32r(Wq[g * GD:(g + 1) * GD, :].T),
            "wk": _round_f32r(wk_g.T),
            "wv": _round_f32r(Wv[g * GD:(g + 1) * GD, :].T),
            "wo": Wo[:, g * GD:(g + 1) * GD].T.astype(ml_dtypes.bfloat16),
            "trigc": trigc[b], "trigs": trigs[b],
            "masks": masks, "onesAb": onesAb, "onesB": _round_f32r(onesB),
            "idb": idb, "idr": _round_f32r(idr),
            "wcolq": wcolq, "wcolk": wcolk,
            "bias8": bias8,
        })
    return in_maps


def kernel(**inputs) -> np.ndarray:
    if "nc" not in _CACHE:
        _CACHE["nc"] = _build()
    nc = _CACHE["nc"]
    in_maps = _host_prep(inputs)
    res = run_bass_kernel_spmd(nc, in_maps, core_ids=list(range(8)))
    out = np.empty((B, T, C), dtype=np.float32)
    for b in range(B):
        out[b] = res.results[2 * b]["out"] + res.results[2 * b + 1]["out"]
    return out


# revision 40
# speedup vs baseline: 1.2241x; 1.2241x over previous
"""Trainium2 Bass kernel for causal self-attention with cumulative-phase rotary
embedding (nn_CausalSelfAttention_64338610094602).

Sharding: 8 cores = 4 batches x 2 head-groups (tensor-parallel over heads).
Each core computes, for its (batch, 8-head group), per head:
  QKV projections, rotation + RMSNorm, causal attention (transposed-scores
  layout, max-free softmax), and a partial output projection. Host sums the
  two head-group partials per batch.

The tiny omega/cumsum/trig path is computed on host (0.016% of FLOPs) and
shipped as cos/sin tables. Score-path GEMMs run in float32r; the probs/V/Wo
side runs in bf16 (rel-err budget ~5e-3 vs 2e-2 tolerance).

Key scheduling ideas vs the previous version:
  - PSUM projection banks are released by a single Pool-engine eviction copy;
    rotation (4 fused DVE ops via duplicated cos/sin tables) reads the SBUF
    copy off the critical path and writes q/k tiles directly (f32r cast).
  - k-side rstd (and the 1/sqrt(D) scale, and both gammas via host-folded
    Wk) ride the exp's per-partition scale, via a PE-transposed rstd column
    table; only the q side needs broadcast-normalize.
  - softmax is software-pipelined 2 blocks deep; denominators are packed
    pairwise (DVE bf16 adds) to halve the PE's ones-matmuls.
  - V is computed d-major at full moving width then PE-transposed to
    token-major bf16.
  - P3 reloads y/Wo (bf16) with DMA prefetched under the last head's
    attention, after the x^T buffer is freed.
"""
import math

import numpy as np
import ml_dtypes

import concourse.mybir as mybir
import concourse.tile as tile
from concourse import bacc
from concourse.bass_utils import run_bass_kernel_spmd

B, T, C = 4, 2048, 2048
H, D, DH = 16, 128, 64
HG = 8          # heads per core (head-group)
GD = HG * D     # group output dims = 1024
NT = T // 512   # 4 J-blocks of 512
NCT = C // 128  # 16 contraction tiles
EPS = 1e-5
SCL = 1.0 / math.sqrt(D)
OMEGA_SCALE = 16.0

dt = mybir.dt
AF = mybir.ActivationFunctionType
ALU = mybir.AluOpType

_CACHE = {}


def _round_f32r(x):
    """Round fp32 array to float32r (13-bit mantissa, round-to-nearest-even)."""
    x = np.ascontiguousarray(x, dtype=np.float32)
    b = x.view(np.uint32).copy()
    low = b & np.uint32(0x3FF)
    bb = b & ~np.uint32(0x3FF)
    rnd = (low > 0x200) | ((low == 0x200) & (((bb >> 10) & 1) == 1))
    return (bb + (rnd.astype(np.uint32) << 10)).view(np.float32)


def _build():
    f32, f32r, bf16 = dt.float32, dt.float32r, dt.bfloat16
    nc = bacc.Bacc(None, target_bir_lowering=False)
    with tile.TileContext(nc) as tc:
        xt_d = nc.dram_tensor("xt", (C, T), f32r, kind="ExternalInput")
        wq_d = nc.dram_tensor("wq", (C, GD), f32r, kind="ExternalInput")
        wk_d = nc.dram_tensor("wk", (C, GD), f32r, kind="ExternalInput")
        wv_d = nc.dram_tensor("wv", (C, GD), f32r, kind="ExternalInput")
        wo_d = nc.dram_tensor("wo", (GD, C), bf16, kind="ExternalInput")
        trigc_d = nc.dram_tensor("trigc", (128, T), f32, kind="ExternalInput")
        trigs_d = nc.dram_tensor("trigs", (128, T), f32, kind="ExternalInput")
        masks_d = nc.dram_tensor("masks", (128, 4 * 512), bf16, kind="ExternalInput")
        onesAb_d = nc.dram_tensor("onesAb", (128, 1), bf16, kind="ExternalInput")
        onesB_d = nc.dram_tensor("onesB", (1, 128), f32r, kind="ExternalInput")
        idb_d = nc.dram_tensor("idb", (128, 128), bf16, kind="ExternalInput")
        idr_d = nc.dram_tensor("idr", (128, 128), f32r, kind="ExternalInput")
        wcolq_d = nc.dram_tensor("wcolq", (128, 7), bf16, kind="ExternalInput")
        wcolk_d = nc.dram_tensor("wcolk", (128, 7), bf16, kind="ExternalInput")
        out_d = nc.dram_tensor("out", (T, C), f32, kind="ExternalOutput")

        with tc.tile_pool(name="dram", bufs=1, space="DRAM") as dramp:
            yspill = dramp.tile([128, 7 * T], bf16)  # heads 0-6 yT

            with tc.tile_pool(name="const", bufs=1) as constp:
                onesAb = constp.tile([128, 1], bf16)
                nc.sync.dma_start(onesAb[:], onesAb_d[:])
                onesB = constp.tile([1, 128], f32r)
                nc.sync.dma_start(onesB[:], onesB_d[:])
                idb = constp.tile([128, 128], bf16)
                nc.sync.dma_start(idb[:], idb_d[:])
                idr = constp.tile([128, 128], f32r)
                nc.sync.dma_start(idr[:], idr_d[:])
                wcolq = constp.tile([128, 7], bf16)
                nc.sync.dma_start(wcolq[:], wcolq_d[:])
                wcolk = constp.tile([128, 7], bf16)
                nc.sync.dma_start(wcolk[:], wcolk_d[:])
                trigc = constp.tile([128, T], f32)
                nc.sync.dma_start(trigc[:], trigc_d[:])
                trigs = constp.tile([128, T], f32)
                nc.sync.dma_start(trigs[:], trigs_d[:])
                masks = constp.tile([128, 4 * 512], bf16)
                nc.sync.dma_start(masks[:], masks_d[:])
                eps4 = constp.tile([4, 1], f32)
                nc.vector.memset(eps4[:], EPS)
                lscl4 = constp.tile([4, 1], f32)
                nc.vector.memset(lscl4[:], math.log(SCL))

                with tc.tile_pool(name="qkv", bufs=1) as qkvp, \
                     tc.tile_pool(name="wst", bufs=6) as wst, \
                     tc.tile_pool(name="wk512", bufs=1) as wkp, \
                     tc.tile_pool(name="exp", bufs=1) as expool, \
                     tc.tile_pool(name="rows", bufs=1) as rowsp:
                    env = dict(
                        nc=nc, tc=tc, wst=wst, wkp=wkp, expool=expool,
                        rowsp=rowsp, qkvp=qkvp, trigc=trigc, trigs=trigs,
                        masks=masks, onesAb=onesAb, onesB=onesB, idb=idb,
                        idr=idr, wcolq=wcolq, wcolk=wcolk,
                        eps4=eps4, lscl4=lscl4,
                        wq_d=wq_d, wk_d=wk_d, wv_d=wv_d, xt_d=xt_d,
                        yspill=yspill)

                    with tc.tile_pool(name="xtp", bufs=1) as xtp:
                        xts = xtp.tile([128, NCT * T], f32r)
                        env["xts"] = xts
                        heads = {}
                        for h in range(HG):
                            heads[h] = _head_qkv(env, h)
                            if h < HG - 1:
                                _head_attn(env, h, heads[h], None)
                    # xts freed; prefetch P3 operands under head 7's attention
                    with tc.tile_pool(name="p3", bufs=1) as p3p, \
                         tc.tile_pool(name="p3o", bufs=1) as p3o:
                        yall = p3p.tile([128, HG * T], bf16)
                        wosb = p3p.tile([128, HG * C], bf16)
                        for h in range(7):
                            nc.sync.dma_start(yall[:, h * T:(h + 1) * T],
                                              yspill[:, h * T:(h + 1) * T])
                        for h in range(HG):
                            nc.sync.dma_start(wosb[:, h * C:(h + 1) * C],
                                              wo_d[h * 128:(h + 1) * 128, :])
                        _head_attn(env, 7, heads[7], yall)

                        with tc.tile_pool(name="p3ps", bufs=1,
                                          space="PSUM") as p3ps:
                            for ti in range(T // 128):
                                ops = [p3ps.tile([128, 512], f32, tag=f"c{cb}",
                                                 bufs=2,
                                                 name=f"op_{ti}_{cb}")
                                       for cb in range(4)]
                                for hh in range(HG):
                                    for cb in range(4):
                                        nc.tensor.matmul(
                                            ops[cb][:],
                                            yall[:, hh * T + ti * 128:
                                                 hh * T + (ti + 1) * 128],
                                            wosb[:, hh * C + cb * 512:
                                                 hh * C + (cb + 1) * 512],
                                            start=(hh == 0), stop=(hh == HG - 1))
                                for cb in range(4):
                                    osb = p3o.tile([128, 512], f32,
                                                   tag=f"osb{cb % 2}",
                                                   bufs=2,
                                                   name=f"osb_{ti}_{cb}")
                                    if cb % 2:
                                        nc.vector.tensor_copy(osb[:], ops[cb][:])
                                    else:
                                        nc.scalar.copy(osb[:], ops[cb][:])
                                    nc.sync.dma_start(
                                        out_d[ti * 128:(ti + 1) * 128,
                                              cb * 512:(cb + 1) * 512],
                                        osb[:])
    nc.compile()
    return nc


def _head_qkv(env, h):
    """Fused V+Q streaming projections, tails, K pass, ssq/rstd for head h."""
    nc = env["nc"]
    tc = env["tc"]
    xts, wst, wkp, qkvp = env["xts"], env["wst"], env["wkp"], env["qkvp"]
    f32, f32r, bf16 = dt.float32, dt.float32r, dt.bfloat16

    q_sb = qkvp.tile([128, T], f32r, tag="q", name=f"q_{h}")
    k_sb = qkvp.tile([128, T], f32r, tag="k", name=f"k_{h}")
    v_sb = qkvp.tile([128, T], bf16, tag="v", name=f"v_{h}")
    vT = qkvp.tile([128, T], bf16, tag="k", name=f"vT_{h}")  # shares k slot
    rstdT = qkvp.tile([128, 16], f32, tag="rT", name=f"rstdT_{h}")

    with tc.tile_pool(name=f"psa_{h}", bufs=1, space="PSUM") as psa:
        # ---- fused stream: V (d-major) + Q projections ride the x DMA ----
        with tc.tile_pool(name=f"psv_{h}", bufs=1, space="PSUM") as psv:
            vps, qps = {}, {}
            for J in range(NT):
                vps[J] = psv.tile([128, 512], f32, tag=f"v{J}",
                                  name=f"vp_{h}_{J}")
                qps[J] = psa.tile([128, 512], f32, tag=f"a{J}",
                                  name=f"qp_{h}_{J}")
            for i in range(NCT):
                if h == 0:
                    nc.sync.dma_start(xts[:, i * T:(i + 1) * T],
                                      env["xt_d"][i * 128:(i + 1) * 128, :])
                wvt = wst.tile([128, 128], f32r, tag="w", name=f"wv_{h}_{i}")
                nc.sync.dma_start(wvt[:],
                                  env["wv_d"][i * 128:(i + 1) * 128,
                                              h * 128:(h + 1) * 128])
                wqt = wst.tile([128, 128], f32r, tag="w", name=f"wq_{h}_{i}")
                nc.sync.dma_start(wqt[:],
                                  env["wq_d"][i * 128:(i + 1) * 128,
                                              h * 128:(h + 1) * 128])
                for J in range(NT):
                    sl = slice(i * T + J * 512, i * T + J * 512 + 512)
                    nc.tensor.matmul(vps[J][:], wvt[:], xts[:, sl],
                                     start=(i == 0), stop=(i == NCT - 1))
                    nc.tensor.matmul(qps[J][:], wqt[:], xts[:, sl],
                                     start=(i == 0), stop=(i == NCT - 1))
            for J in range(NT):
                nc.scalar.copy(vT[:, J * 512:(J + 1) * 512], vps[J][:])
        # ---- q tails: evict (DVE), square (ACT, bf16), rotate ----
        sqq = []
        qtmps = []
        for J in range(NT):
            qtmp = wkp.tile([128, 512], f32, tag=f"qt{J}", name=f"qtmp_{h}_{J}")
            nc.vector.tensor_copy(qtmp[:], qps[J][:])  # frees bank a{J}
            qtmps.append(qtmp)
        for J in range(NT):
            sq = wkp.tile([128, 512], bf16, tag=f"sq{J}", name=f"sq_{h}_{J}")
            nc.scalar.activation(sq[:], qtmps[J][:], AF.Square)
            sqq.append(sq)
            _rotate(env, qtmps[J], q_sb, J)
        # ---- V transposes to token-major bf16 ----
        with tc.tile_pool(name=f"pst_{h}", bufs=2, space="PSUM") as pst:
            for tq in range(4):
                tp = pst.tile([128, 512], bf16, tag="tp", name=f"tp_{h}_{tq}")
                for k in range(4):
                    tt = tq * 4 + k
                    nc.tensor.transpose(tp[:, k * 128:(k + 1) * 128],
                                        vT[:, tt * 128:(tt + 1) * 128],
                                        env["idb"][:])
                nc.scalar.copy(v_sb[:, tq * 512:(tq + 1) * 512], tp[:])
        # ---- K pass (reuses the a{J} banks); rstd_q mid-stream ----
        with tc.tile_pool(name=f"pss_{h}", bufs=1, space="PSUM") as pss:
            ssq_q = pss.tile([4, 512], f32, tag="ssq_q", name=f"ssqq_{h}")
            ssq_k = pss.tile([4, 512], f32, tag="ssq_k", name=f"ssqk_{h}")
            kps = {}
            for J in range(NT):
                kps[J] = psa.tile([128, 512], f32, tag=f"a{J}",
                                  name=f"kp_{h}_{J}")

            def kchunk(lo, hi):
                for i in range(lo, hi):
                    wkt = wst.tile([128, 128], f32r, tag="w",
                                   name=f"wk_{h}_{i}")
                    nc.sync.dma_start(wkt[:],
                                      env["wk_d"][i * 128:(i + 1) * 128,
                                                  h * 128:(h + 1) * 128])
                    for J in range(NT):
                        nc.tensor.matmul(
                            kps[J][:], wkt[:],
                            xts[:, i * T + J * 512:i * T + J * 512 + 512],
                            start=(i == 0), stop=(i == NCT - 1))

            kchunk(0, 8)
            # q ssq + rstd_q: squares done by now; hides under the K pass
            for J in range(NT):
                nc.tensor.matmul(ssq_q[:], env["wcolq"][:, 3 - J:7 - J],
                                 sqq[J][:], start=(J == 0), stop=(J == NT - 1))
            lnq = wkp.tile([4, 512], f32, tag="tcc", name=f"lnq_{h}")
            nc.scalar.activation(lnq[:], ssq_q[:], AF.Ln, scale=1.0 / 128.0,
                                 bias=env["eps4"][:])
            rstd_q = wkp.tile([4, 512], f32r, tag="rstd8", name=f"rstdq_{h}")
            nc.scalar.activation(rstd_q[:], lnq[:], AF.Exp, scale=-0.5)
            kchunk(8, NCT)
            # k tails: evict all banks first, square+ssq, rotations deferred
            ktmps = []
            for J in range(NT):
                ktmp = wkp.tile([128, 512], f32, tag=f"qt{J}",
                                name=f"ktmp_{h}_{J}")
                nc.vector.tensor_copy(ktmp[:], kps[J][:])
                ktmps.append(ktmp)
            for J in range(NT):
                sqk = wkp.tile([128, 512], bf16, tag=f"sq{J}",
                               name=f"sqk_{h}_{J}")
                nc.scalar.activation(sqk[:], ktmps[J][:], AF.Square)
                nc.tensor.matmul(ssq_k[:], env["wcolk"][:, 3 - J:7 - J],
                                 sqk[:], start=(J == 0), stop=(J == NT - 1))
            lnk = wkp.tile([4, 512], f32, tag="tcc", name=f"lnk_{h}")
            nc.scalar.activation(lnk[:], ssq_k[:], AF.Ln, scale=1.0 / 128.0,
                                 bias=env["eps4"][:])
            # fold the 1/sqrt(D) softmax scale into the k-side rstd
            rstd_k = wkp.tile([4, 512], f32r, tag="tss", name=f"rstdk_{h}")
            nc.scalar.activation(rstd_k[:], lnk[:], AF.Exp, scale=-0.5,
                                 bias=env["lscl4"][:])
            # transpose k rows into per-key columns for the exp scale
            with tc.tile_pool(name=f"psr_{h}", bufs=1, space="PSUM") as psr:
                rt = psr.tile([128, 16], f32r, name=f"rt_{h}")
                for c in range(4):
                    nc.tensor.transpose(rt[:, c * 4:(c + 1) * 4],
                                        rstd_k[0:4, c * 128:(c + 1) * 128],
                                        env["idr"][0:4, 0:4])
                nc.vector.tensor_copy(rstdT[:], rt[:])

    # k rotations deferred into the attention J-blocks
    krots = [
        (lambda J=J, ktmp=ktmps[J]: _rotate(env, ktmp, k_sb, J))
        for J in range(NT)
    ]
    return q_sb, k_sb, v_sb, rstdT, rstd_q, krots


def _rotate(env, qtmp, dest, J):
    """rot1 = q1 cos + q2 sin ; rot2 = q2 cos - q1 sin.

    DVE inputs must share a start partition; the cross-half sin products
    ride the (legal) shifted writes of tsw. The sub runs on Pool."""
    nc = env["nc"]
    wkp = env["wkp"]
    f32 = dt.float32
    sl = slice(J * 512, (J + 1) * 512)
    tcc = wkp.tile([128, 512], f32, tag="tcc", name="tcc_t")
    tsw = wkp.tile([128, 512], f32, tag="tss", name="tsw_t")
    nc.vector.tensor_tensor(tcc[:], qtmp[:], env["trigc"][:, sl],
                            op=ALU.mult)
    nc.vector.tensor_tensor(tsw[0:DH, :], qtmp[DH:128, :],
                            env["trigs"][DH:128, sl], op=ALU.mult)
    nc.vector.tensor_tensor(tsw[DH:128, :], qtmp[0:DH, :],
                            env["trigs"][0:DH, sl], op=ALU.mult)
    nc.vector.tensor_add(dest[0:DH, sl], tcc[0:DH, :], tsw[0:DH, :])
    nc.gpsimd.tensor_sub(dest[DH:128, sl], tcc[DH:128, :], tsw[DH:128, :])


def _head_attn(env, h, qkv, yall):
    """2c: causal attention for head h (transposed scores, pipelined)."""
    nc = env["nc"]
    tc = env["tc"]
    q_sb, k_sb, v_sb, rstdT, rstd_q, krots = qkv
    expool, wkp, rowsp = env["expool"], env["wkp"], env["rowsp"]
    masks, onesAb, onesB = env["masks"], env["onesAb"], env["onesB"]
    f32, f32r, bf16 = dt.float32, dt.float32r, dt.bfloat16

    with tc.tile_pool(name=f"psc_{h}", bufs=1, space="PSUM") as psc:
        pend = [None]

        def epilogue(yps, dps, J):
            rcf = wkp.tile([1, 512], f32, tag="tcc", name=f"rcf_{h}_{J}")
            nc.vector.reciprocal_approx_fast(out=rcf[:], in_=dps[:])
            recip = wkp.tile([1, 512], f32r, tag="tss", name=f"recip_{h}_{J}")
            nc.vector.tensor_copy(recip[:], rcf[:])
            rbps = psc.tile([128, 512], f32, tag="rb", bufs=1,
                            name=f"rbps_{h}_{J}")
            nc.tensor.matmul(rbps[:], onesB[:], recip[:], start=True, stop=True)
            rbsb = wkp.tile([128, 512], f32, tag="rbsb", name=f"rbsb_{h}_{J}")
            nc.vector.tensor_copy(rbsb[:], rbps[:])
            sl = slice(J * 512, (J + 1) * 512)
            if yall is None:
                yt = expool.tile([128, 512], bf16, tag="yt", name=f"yt_{h}_{J}")
                nc.vector.tensor_tensor(yt[:], yps[:], rbsb[:], op=ALU.mult)
                nc.sync.dma_start(
                    env["yspill"][:, h * T + J * 512:h * T + (J + 1) * 512],
                    yt[:])
            else:
                nc.vector.tensor_tensor(yall[:, 7 * T + J * 512:
                                             7 * T + (J + 1) * 512],
                                        yps[:], rbsb[:], op=ALU.mult)

        for J in range(NT):
            nI = 4 * J + 4
            # deferred k rotation for this key range
            krots[J]()
            # JIT q-normalize for this J (broadcast rstd row over partitions)
            sl = slice(J * 512, (J + 1) * 512)
            rrow = rowsp.tile([1, 512], f32r, tag="r0", name=f"rrow_{h}_{J}")
            nc.sync.dma_start(rrow[:], rstd_q[J:J + 1, :])
            rbn = psc.tile([128, 512], f32, tag="rb", bufs=1,
                           name=f"rbn_{h}_{J}")
            nc.tensor.matmul(rbn[:], onesB[:], rrow[:], start=True, stop=True)
            nc.vector.tensor_tensor(q_sb[:, sl], q_sb[:, sl], rbn[:],
                                    op=ALU.mult)

            yps = psc.tile([128, 512], f32, tag="y", bufs=2,
                           name=f"yps_{h}_{J}")
            dps = psc.tile([1, 512], f32, tag="d", bufs=2,
                           name=f"dps_{h}_{J}")
            uses = []
            pairs = []
            quads = []

            def emit_y(I, yps=yps, dps=dps, nI=nI, uses=uses, quads=quads):
                nc.tensor.matmul(
                    yps[:], v_sb[:, I * 128:(I + 1) * 128], uses[I][:],
                    start=(I == 0), stop=(I == nI - 1))
                if I % 4 == 3:
                    m = I // 4
                    nc.tensor.matmul(dps[:], onesAb[:], quads[m][:],
                                     start=(m == 0), stop=(m == nI // 4 - 1))

            for I in range(nI):
                sps = psc.tile([128, 512], f32, tag="s", bufs=3,
                               name=f"sps_{h}_{J}_{I}")
                nc.tensor.matmul(
                    sps[:], k_sb[:, I * 128:(I + 1) * 128],
                    q_sb[:, J * 512:(J + 1) * 512], start=True, stop=True)
                ex = expool.tile([128, 512], bf16, tag=f"ex{I % 3}",
                                 name=f"ex_{h}_{J}_{I}")
                col = (I % 4) * 4 + (I // 4)
                nc.scalar.activation(ex[:], sps[:], AF.Exp,
                                     scale=rstdT[:, col:col + 1])
                if I >= 4 * J:
                    r = I - 4 * J
                    exm = expool.tile([128, 512], bf16, tag=f"mk{I % 3}",
                                      name=f"exm_{h}_{J}_{I}")
                    nc.vector.tensor_tensor(
                        exm[:], ex[:], masks[:, r * 512:(r + 1) * 512],
                        op=ALU.mult)
                    uses.append(exm)
                else:
                    uses.append(ex)
                if I % 2 == 1:
                    pm = expool.tile([128, 512], bf16, tag=f"ps{(I // 2) % 2}",
                                     name=f"pm_{h}_{J}_{I}")
                    nc.vector.tensor_add(pm[:], uses[I - 1][:], uses[I][:])
                    pairs.append(pm)
                if I % 4 == 3:
                    qd = expool.tile([128, 512], bf16, tag=f"qd{(I // 4) % 2}",
                                     name=f"qd_{h}_{J}_{I}")
                    nc.vector.tensor_add(qd[:], pairs[I // 2 - 1][:],
                                         pairs[I // 2][:])
                    quads.append(qd)
                if I == 1 and pend[0] is not None:
                    pend[0]()
                    pend[0] = None
                if I >= 2:
                    emit_y(I - 2)
            emit_y(nI - 2)
            emit_y(nI - 1)
            pend[0] = (lambda yps=yps, dps=dps, J=J: epilogue(yps, dps, J))
        pend[0]()
        pend[0] = None


def _host_prep(inputs):
    x = np.asarray(inputs["x"], dtype=np.float32)
    Wq = np.asarray(inputs["Wq"], dtype=np.float32)
    Wk = np.asarray(inputs["Wk"], dtype=np.float32)
    Wv = np.asarray(inputs["Wv"], dtype=np.float32)
    Wo = np.asarray(inputs["Wo"], dtype=np.float32)
    w_omega = np.asarray(inputs["w_omega"], dtype=np.float32)
    b_omega = np.asarray(inputs["b_omega"], dtype=np.float32)
    log_freq = np.asarray(inputs["log_freq"], dtype=np.float32)
    q_gamma = np.asarray(inputs["q_gamma"], dtype=np.float32)
    k_gamma = np.asarray(inputs["k_gamma"], dtype=np.float32)

    # host trig path (tiny): omega -> phi -> cos/sin tables per batch
    z = (x.reshape(B * T, C) @ w_omega.reshape(C).astype(np.float64)
         ).reshape(B, T) + float(b_omega[0])
    omega = 1.0 / (1.0 + np.exp(-z.astype(np.float64) / OMEGA_SCALE))
    phi = np.cumsum(omega, axis=1) - omega                       # (B,T)
    freq = np.exp(log_freq.astype(np.float64))                   # (DH,)
    ang = phi[:, None, :] * freq[:, None]                        # (B,DH,T)
    cosb = np.cos(ang).astype(np.float32)
    sinb = np.sin(ang).astype(np.float32)
    trigc = np.concatenate([cosb, cosb], axis=1)                 # (B,128,T)
    trigs = np.concatenate([sinb, sinb], axis=1)

    g2 = (q_gamma * k_gamma).astype(np.float32)                  # (128,)
    wk_scale = np.tile(g2, HG)                                   # (GD per group)
    inv_g2sq = np.zeros(128, dtype=np.float32)
    nz = np.abs(g2) > 1e-12
    inv_g2sq[nz] = 1.0 / (g2[nz] * g2[nz])

    p = np.arange(128)[:, None]
    c = np.arange(512)[None, :]
    masks = np.concatenate(
        [((p + r * 128) <= c).astype(np.float32) for r in range(4)], axis=1
    ).astype(ml_dtypes.bfloat16)
    onesAb = np.ones((128, 1), dtype=ml_dtypes.bfloat16)
    onesB = np.ones((1, 128), dtype=np.float32)
    idb = np.eye(128, dtype=ml_dtypes.bfloat16)
    idr = np.eye(128, dtype=np.float32)
    wcolq = np.zeros((128, 7), dtype=np.float32)
    wcolq[:, 3] = 1.0
    wcolq = wcolq.astype(ml_dtypes.bfloat16)
    wcolk = np.zeros((128, 7), dtype=np.float32)
    wcolk[:, 3] = inv_g2sq
    wcolk = wcolk.astype(ml_dtypes.bfloat16)

    in_maps = []
    for core in range(8):
        b, g = core // 2, core % 2
        wk_g = Wk[g * GD:(g + 1) * GD, :] * wk_scale[:, None]
        in_maps.append({
            "xt": _round_f32r(x[b].T),
            "wq": _round_f32r(Wq[g * GD:(g + 1) * GD, :].T),
            "wk": _round_f32r(wk_g.T),
            "wv": _round_f32r(Wv[g * GD:(g + 1) * GD, :].T),
            "wo": Wo[:, g * GD:(g + 1) * GD].T.astype(ml_dtypes.bfloat16),
            "trigc": trigc[b], "trigs": trigs[b],
            "masks": masks, "onesAb": onesAb, "onesB": _round_f32r(onesB),
            "idb": idb, "idr": _round_f32r(idr),
            "wcolq": wcolq, "wcolk": wcolk,
        })
    return in_maps


def kernel(**inputs) -> np.ndarray:
    if "nc" not in _CACHE:
        _CACHE["nc"] = _build()
    nc = _CACHE["nc"]
    in_maps = _host_prep(inputs)
    res = run_bass_kernel_spmd(nc, in_maps, core_ids=list(range(8)))
    out = np.empty((B, T, C), dtype=np.float32)
    for b in range(B):
        out[b] = res.results[2 * b]["out"] + res.results[2 * b + 1]["out"]
    return out
